# revision 37
# baseline (speedup 1.0000x reference)
"""24x24-bit array multiplier on 8 TRN2 NeuronCores (Bass).

A, B: [65536, 24] float32 0/1 bit-vectors (LSB first) -> P: [65536, 48] bits.

Strategy (pure data parallel, batch sharded 8 ways):
  Per core shard = 8192 rows laid out as [128 partitions x 64 rows/partition],
  processed in chunks pipelined across engines:
  1. bits -> 12-bit limb values via a fused shift-add tree
     (scalar_tensor_tensor), A and B side by side in one tile so each
     tree level is a single op; all fp32 exact (values < 4096). [VectorE]
  2. All 4 limb cross-products in one op via repeat-APs, fp32
     exact (< 2^24); cast to int32.                      [VectorE, ScalarE]
  3. Radix-4096 digit-serial combine: shifts/masks on VectorE (the
     only engine walrus allows TensorScalarPtr on), carry adds on
     GpSimd (plain TT adds are Pool-legal). Every add < 2^14 because
     the vector engines run int adds through the fp32 datapath
     (exact only below 2^24); "mod 4096" is the fused arithmetic
     (x>>12)*-4096 + x, exact in fp32.
  4. All 48 bits at once: digits broadcast-ANDed against an
     iota-built 12-mask vector [VectorE], then Sign() -> fp32 0/1
     [ScalarE], DMA out.
"""

import numpy as np

import concourse.bacc as bacc
import concourse.bass as bass
import concourse.mybir as mybir
import concourse.tile as tile
from concourse.alu_op_type import AluOpType
from concourse.bass_utils import run_bass_kernel_spmd

P = 128           # SBUF partitions
C = 64            # batch rows per partition
NB = P * C        # rows per core = 8192
N_CORES = 8
BATCH = NB * N_CORES

f32 = mybir.dt.float32
i32 = mybir.dt.int32

AND, SHR, SHL = (AluOpType.bitwise_and, AluOpType.logical_shift_right,
                 AluOpType.logical_shift_left)
ADD, OR, MUL = AluOpType.add, AluOpType.bitwise_or, AluOpType.mult
SIGN = mybir.ActivationFunctionType.Sign

DEFAULT_BUILD = dict(chunk_sizes=[16, 24, 16, 8], bufs=3, skew=3,
                     out_eng="sync", preload=True)


def build_nc(chunk_sizes: list | None = None, bufs: int = 3,
             cast_eng: str = "scalar", adds_eng: str = "gpsimd",
             sign_eng: str = "scalar", out_eng: str = "scalar",
             d0_eng: str = "scalar", skew: int = 3, preload: bool = False,
             chunk_overrides: dict | None = None) -> bass.Bass:
    """One SPMD program; every core runs it on its own shard."""
    nc = bacc.Bacc(
        "TRN2",
        target_bir_lowering=False,
        debug=False,
        num_devices=N_CORES,
    )
    A = nc.declare_dram_parameter("A", [NB, 24], f32, isOutput=False)
    B = nc.declare_dram_parameter("B", [NB, 24], f32, isOutput=False)
    OUT = nc.declare_dram_parameter("out", [NB, 48], f32, isOutput=True)

    if chunk_sizes is None:
        chunk_sizes = DEFAULT_BUILD["chunk_sizes"]
    assert sum(chunk_sizes) == C

    # DRAM views: row (p*C + c) lives on partition p, slot c.
    Av = A[:].rearrange("(p c) b -> p c b", p=P)
    Bv = B[:].rearrange("(p c) b -> p c b", p=P)
    Ov = OUT[:].rearrange("(p c) b -> p c b", p=P)

    with tile.TileContext(nc) as tc, \
            tc.tile_pool(name="const", bufs=1) as cpool, \
            tc.tile_pool(name="work", bufs=bufs) as pool:
        # mask[p, k] = 1 << k for k in 0..11, built on-chip (no DMA)
        iot = cpool.tile([P, 12], i32)
        nc.gpsimd.iota(iot[:], [[1, 12]], channel_multiplier=0)
        ones = cpool.tile([P, 12], i32)
        nc.vector.memset(ones[:], 1)
        mask_t = cpool.tile([P, 12], i32)
        nc.vector.tensor_tensor(mask_t[:], ones[:], iot[:], SHL)
        mask_r = mask_t[:].rearrange("p b -> p () () b")

        def make_stages(ch, cc, row0):
            """Return the list of stage-emitter closures for one chunk."""
            ov = (chunk_overrides or {}).get(ch, {})
            c_cast = ov.get("cast_eng", cast_eng)
            c_adds = ov.get("adds_eng", adds_eng)
            c_sign = ov.get("sign_eng", sign_eng)
            c_out = ov.get("out_eng", out_eng)
            c_d0 = ov.get("d0_eng", d0_eng)
            rows = slice(row0, row0 + cc)
            mask_bc = mask_r.broadcast_to((P, cc, 4, 12))
            v_ = nc.vector
            g = getattr(nc, c_adds)   # plain adds (Pool-legal)
            S: dict = {}

            def scratch(tg, k=1):
                s = pool.tile([P, cc * k], i32, tag=tg)
                return s[:].rearrange("p (c k) -> p c k", k=k)

            def s_load():
                # A and B side by side: [P, (2, cc, 24)]
                ab = pool.tile([P, 2 * cc * 24], f32, tag="ab")
                S["ab3"] = ab[:].rearrange("p (i c b) -> p i c b", i=2, b=24)
                nc.sync.dma_start(S["ab3"][:, 0], Av[:, rows])
                nc.sync.dma_start(S["ab3"][:, 1], Bv[:, rows])

            def s_limbs():
                # shift-add tree -> limbs [P, (2, cc, 2)] fp32 (lo12, hi12)
                ab3 = S["ab3"]
                u = pool.tile([P, 2 * cc * 12], f32, tag="u")
                u3 = u[:].rearrange("p (i c b) -> p i c b", i=2, b=12)
                v_.scalar_tensor_tensor(u3, ab3[:, :, :, 1::2], 2.0,
                                        ab3[:, :, :, 0::2], MUL, ADD)
                v = pool.tile([P, 2 * cc * 6], f32, tag="v")
                v3 = v[:].rearrange("p (i c b) -> p i c b", i=2, b=6)
                v_.scalar_tensor_tensor(v3, u3[:, :, :, 1::2], 4.0,
                                        u3[:, :, :, 0::2], MUL, ADD)
                v4 = v[:].rearrange("p (i c l b) -> p i c l b", i=2, l=2, b=3)
                t = pool.tile([P, 2 * cc * 2], f32, tag="t")
                t4 = t[:].rearrange("p (i c l) -> p i c l", i=2, l=2)
                v_.scalar_tensor_tensor(t4, v4[:, :, :, :, 1], 16.0,
                                        v4[:, :, :, :, 0], MUL, ADD)
                lm = pool.tile([P, 2 * cc * 2], f32, tag="lm")
                S["lm4"] = lm[:].rearrange("p (i c l) -> p i c l", i=2, l=2)
                v_.scalar_tensor_tensor(S["lm4"], v4[:, :, :, :, 2], 256.0,
                                        t4, MUL, ADD)

            def s_prod():
                # all 4 cross products in one op:
                # pt[c, :] = (a0b0, a0b1, a1b0, a1b1)
                lm4 = S["lm4"]
                la_rep = lm4[:, 0].rearrange("p c l -> p c l ()").broadcast_to(
                    (P, cc, 2, 2))                  # (a0,a0,a1,a1)
                lb_rep = lm4[:, 1].rearrange("p c l -> p c () l").broadcast_to(
                    (P, cc, 2, 2))                  # (b0,b1,b0,b1)
                pt = pool.tile([P, cc * 4], f32, tag="pt")
                pt4 = pt[:].rearrange("p (c i l) -> p c i l", i=2, l=2)
                v_.tensor_tensor(pt4, la_rep, lb_rep, MUL)
                S["pt"] = pt

            def s_cast():
                it = pool.tile([P, cc * 4], i32, tag="it")
                if c_cast == "scalar":
                    nc.scalar.copy(it[:], S["pt"][:])  # fp32 -> int32 exact
                else:
                    getattr(nc, c_cast).tensor_copy(it[:], S["pt"][:])
                S["it3"] = it[:].rearrange("p (c k) -> p c k", k=4)

            def s_shq():
                sh = scratch("sh", 4)     # pXY >> 12
                v_.tensor_scalar(sh, S["it3"], 12, None, SHR)
                q = scratch("q", 4)       # pXY mod 4096
                v_.scalar_tensor_tensor(q, sh, -4096.0, S["it3"], MUL, ADD)
                S["sh"], S["q"] = sh, q
                dig = scratch("dig", 4)   # the four 12-bit output digits
                S["dig"] = dig
                if c_d0 == "scalar":
                    nc.scalar.copy(dig[:, :, 0:1], q[:, :, 0:1])
                else:
                    getattr(nc, c_d0).tensor_copy(dig[:, :, 0:1], q[:, :, 0:1])

            def s_add1():
                sh, q = S["sh"], S["q"]
                t1 = scratch("t1")
                g.tensor_tensor(t1, q[:, :, 1:2], q[:, :, 2:3], ADD)
                g.tensor_tensor(t1, t1, sh[:, :, 0:1], ADD)   # digit1 raw <= 12283
                u2 = scratch("u2")
                g.tensor_tensor(u2, sh[:, :, 1:2], sh[:, :, 2:3], ADD)
                g.tensor_tensor(u2, u2, q[:, :, 3:4], ADD)
                S["t1"], S["u2"] = t1, u2

            def s_c1():
                c1 = scratch("c1")
                v_.tensor_scalar(c1, S["t1"], 12, None, SHR)
                v_.scalar_tensor_tensor(S["dig"][:, :, 1:2], c1, -4096.0,
                                        S["t1"], MUL, ADD)
                S["c1"] = c1

            def s_add2():
                t2 = scratch("t2")
                g.tensor_tensor(t2, S["u2"], S["c1"], ADD)    # digit2 raw <= 12285
                S["t2"] = t2

            def s_c2():
                c2 = scratch("c2")
                v_.tensor_scalar(c2, S["t2"], 12, None, SHR)
                v_.scalar_tensor_tensor(S["dig"][:, :, 2:3], c2, -4096.0,
                                        S["t2"], MUL, ADD)
                S["c2"] = c2

            def s_d3():
                g.tensor_tensor(S["dig"][:, :, 3:4], S["sh"][:, :, 3:4],
                                S["c2"], ADD)                 # digit3 <= 4095

            def s_band():
                # bits: (digit & (1<<k)); out position = 12*digit + k,
                # contiguous in (c, digit, bit).
                bt = pool.tile([P, cc * 48], i32, tag="bt")
                bt4 = bt[:].rearrange("p (c l b) -> p c l b", l=4, b=12)
                v_.tensor_tensor(bt4, S["dig"].broadcast_to((P, cc, 4, 12)),
                                 mask_bc, AND)
                S["bt"] = bt

            def s_sign():
                ob = pool.tile([P, cc * 48], f32, tag="ob")
                if c_sign == "scalar":
                    nc.scalar.activation(ob[:], S["bt"][:], SIGN)
                else:
                    nc.vector.tensor_scalar(ob[:], S["bt"][:], 0, None,
                                            AluOpType.not_equal)
                S["ob"] = ob

            def s_out():
                # out-DMA off SP: a single engine's DMAs issue in order, so
                # an output blocked on compute would head-of-line-block
                # later loads.
                getattr(nc, c_out).dma_start(
                    Ov[:, rows], S["ob"][:].rearrange("p (c b) -> p c b", b=48))

            return [s_load, s_limbs, s_prod, s_cast, s_shq, s_add1, s_c1,
                    s_add2, s_c2, s_d3, s_band, s_sign, s_out]

        # software-pipelined emission: engine streams are executed in
        # priority (= emission) order, so interleave chunks with a skew to
        # give every engine independent work during cross-engine stalls.
        chunk_stages = []
        row0 = 0
        for ch, cc in enumerate(chunk_sizes):
            chunk_stages.append(make_stages(ch, cc, row0))
            row0 += cc
        n_stages = len(chunk_stages[0])
        n_ch = len(chunk_stages)
        if preload:
            for k in range(n_ch):
                chunk_stages[k][0]()
        for t in range(n_stages + (n_ch - 1) * skew):
            for k in range(n_ch):
                s = t - k * skew
                if (1 if preload else 0) <= s < n_stages:
                    chunk_stages[k][s]()

    nc.compile()
    return nc


_CACHE: dict = {}


def kernel(A: np.ndarray, B: np.ndarray) -> np.ndarray:
    A = np.ascontiguousarray(np.asarray(A, dtype=np.float32))
    B = np.ascontiguousarray(np.asarray(B, dtype=np.float32))
    assert A.shape == (BATCH, 24) and B.shape == (BATCH, 24), (A.shape, B.shape)

    if "nc" not in _CACHE:
        _CACHE["nc"] = build_nc(**DEFAULT_BUILD)
    nc = _CACHE["nc"]

    in_maps = []
    for i in range(N_CORES):
        sl = slice(i * NB, (i + 1) * NB)
        in_maps.append({"A": A[sl], "B": B[sl]})

    res = run_bass_kernel_spmd(nc, in_maps, core_ids=list(range(N_CORES)))
    outs = [np.asarray(res.results[i]["out"]) for i in range(N_CORES)]
    return np.concatenate(outs, axis=0).astype(np.float32)


if __name__ == "__main__":
    rng = np.random.default_rng(0)
    A = rng.integers(0, 2, (BATCH, 24)).astype(np.float32)
    B = rng.integers(0, 2, (BATCH, 24)).astype(np.float32)
    out = kernel(A, B)
    pw = (1 << np.arange(24)).astype(np.int64)
    a = (A.astype(np.int64) * pw).sum(-1)
    b = (B.astype(np.int64) * pw).sum(-1)
    p = a * b
    exp = ((p[:, None] >> np.arange(48)[None, :]) & 1).astype(np.float32)
    print("max abs diff:", np.abs(out - exp).max())
    assert np.array_equal(out, exp), "MISMATCH"
    print("EXACT MATCH")


# revision 38
# speedup vs baseline: 1.0408x; 1.0408x over previous
"""24x24-bit array multiplier on 8 TRN2 NeuronCores (Bass).

A, B: [65536, 24] float32 0/1 bit-vectors (LSB first) -> P: [65536, 48] bits.

Strategy (pure data parallel, batch sharded 8 ways):
  Per core shard = 8192 rows laid out as [128 partitions x 64 rows/partition],
  processed in chunks pipelined across engines:
  1. bits -> 12-bit limb values via a fused shift-add tree
     (scalar_tensor_tensor), A and B side by side in one tile so each
     tree level is a single op; all fp32 exact (values < 4096). [VectorE]
  2. All 4 limb cross-products in one op via repeat-APs, fp32
     exact (< 2^24); cast to int32.                      [VectorE, ScalarE]
  3. Radix-4096 digit-serial combine: shifts/masks on VectorE (the
     only engine walrus allows TensorScalarPtr on), carry adds on
     GpSimd (plain TT adds are Pool-legal). Every add < 2^14 because
     the vector engines run int adds through the fp32 datapath
     (exact only below 2^24); "mod 4096" is the fused arithmetic
     (x>>12)*-4096 + x, exact in fp32.
  4. All 48 bits at once: digits broadcast-ANDed against an
     iota-built 12-mask vector [VectorE], then Sign() -> fp32 0/1
     [ScalarE], DMA out.
"""

import numpy as np

import concourse.bacc as bacc
import concourse.bass as bass
import concourse.mybir as mybir
import concourse.tile as tile
from concourse.alu_op_type import AluOpType
from concourse.bass_utils import run_bass_kernel_spmd

P = 128           # SBUF partitions
C = 64            # batch rows per partition
NB = P * C        # rows per core = 8192
N_CORES = 8
BATCH = NB * N_CORES

f32 = mybir.dt.float32
i32 = mybir.dt.int32

AND, SHR, SHL = (AluOpType.bitwise_and, AluOpType.logical_shift_right,
                 AluOpType.logical_shift_left)
ADD, OR, MUL = AluOpType.add, AluOpType.bitwise_or, AluOpType.mult
SIGN = mybir.ActivationFunctionType.Sign

DEFAULT_BUILD = dict(chunk_sizes=[20, 20, 16, 8], bufs=3, skew=3,
                     out_eng="sync", preload=True, cast_eng="gpsimd")


def build_nc(chunk_sizes: list | None = None, bufs: int = 3,
             cast_eng: str = "scalar", adds_eng: str = "gpsimd",
             sign_eng: str = "scalar", out_eng: str = "scalar",
             d0_eng: str = "scalar", skew: int = 3, preload: bool = False,
             chunk_overrides: dict | None = None) -> bass.Bass:
    """One SPMD program; every core runs it on its own shard."""
    nc = bacc.Bacc(
        "TRN2",
        target_bir_lowering=False,
        debug=False,
        num_devices=N_CORES,
    )
    A = nc.declare_dram_parameter("A", [NB, 24], f32, isOutput=False)
    B = nc.declare_dram_parameter("B", [NB, 24], f32, isOutput=False)
    OUT = nc.declare_dram_parameter("out", [NB, 48], f32, isOutput=True)

    if chunk_sizes is None:
        chunk_sizes = DEFAULT_BUILD["chunk_sizes"]
    assert sum(chunk_sizes) == C

    # DRAM views: row (p*C + c) lives on partition p, slot c.
    Av = A[:].rearrange("(p c) b -> p c b", p=P)
    Bv = B[:].rearrange("(p c) b -> p c b", p=P)
    Ov = OUT[:].rearrange("(p c) b -> p c b", p=P)

    with tile.TileContext(nc) as tc, \
            tc.tile_pool(name="const", bufs=1) as cpool, \
            tc.tile_pool(name="work", bufs=bufs) as pool:
        # mask[p, k] = 1 << k for k in 0..11, built on-chip (no DMA)
        iot = cpool.tile([P, 12], i32)
        nc.gpsimd.iota(iot[:], [[1, 12]], channel_multiplier=0)
        ones = cpool.tile([P, 12], i32)
        nc.vector.memset(ones[:], 1)
        mask_t = cpool.tile([P, 12], i32)
        nc.vector.tensor_tensor(mask_t[:], ones[:], iot[:], SHL)
        mask_r = mask_t[:].rearrange("p b -> p () () b")

        def make_stages(ch, cc, row0):
            """Return the list of stage-emitter closures for one chunk."""
            ov = (chunk_overrides or {}).get(ch, {})
            c_cast = ov.get("cast_eng", cast_eng)
            c_adds = ov.get("adds_eng", adds_eng)
            c_sign = ov.get("sign_eng", sign_eng)
            c_out = ov.get("out_eng", out_eng)
            c_d0 = ov.get("d0_eng", d0_eng)
            rows = slice(row0, row0 + cc)
            mask_bc = mask_r.broadcast_to((P, cc, 4, 12))
            v_ = nc.vector
            g = getattr(nc, c_adds)   # plain adds (Pool-legal)
            S: dict = {}

            def scratch(tg, k=1):
                s = pool.tile([P, cc * k], i32, tag=tg)
                return s[:].rearrange("p (c k) -> p c k", k=k)

            def s_load():
                # A and B side by side: [P, (2, cc, 24)]
                ab = pool.tile([P, 2 * cc * 24], f32, tag="ab")
                S["ab3"] = ab[:].rearrange("p (i c b) -> p i c b", i=2, b=24)
                nc.sync.dma_start(S["ab3"][:, 0], Av[:, rows])
                nc.sync.dma_start(S["ab3"][:, 1], Bv[:, rows])

            def s_limbs():
                # shift-add tree -> limbs [P, (2, cc, 2)] fp32 (lo12, hi12)
                ab3 = S["ab3"]
                u = pool.tile([P, 2 * cc * 12], f32, tag="u")
                u3 = u[:].rearrange("p (i c b) -> p i c b", i=2, b=12)
                v_.scalar_tensor_tensor(u3, ab3[:, :, :, 1::2], 2.0,
                                        ab3[:, :, :, 0::2], MUL, ADD)
                v = pool.tile([P, 2 * cc * 6], f32, tag="v")
                v3 = v[:].rearrange("p (i c b) -> p i c b", i=2, b=6)
                v_.scalar_tensor_tensor(v3, u3[:, :, :, 1::2], 4.0,
                                        u3[:, :, :, 0::2], MUL, ADD)
                v4 = v[:].rearrange("p (i c l b) -> p i c l b", i=2, l=2, b=3)
                t = pool.tile([P, 2 * cc * 2], f32, tag="t")
                t4 = t[:].rearrange("p (i c l) -> p i c l", i=2, l=2)
                v_.scalar_tensor_tensor(t4, v4[:, :, :, :, 1], 16.0,
                                        v4[:, :, :, :, 0], MUL, ADD)
                lm = pool.tile([P, 2 * cc * 2], f32, tag="lm")
                S["lm4"] = lm[:].rearrange("p (i c l) -> p i c l", i=2, l=2)
                v_.scalar_tensor_tensor(S["lm4"], v4[:, :, :, :, 2], 256.0,
                                        t4, MUL, ADD)

            def s_prod():
                # all 4 cross products in one op:
                # pt[c, :] = (a0b0, a0b1, a1b0, a1b1)
                lm4 = S["lm4"]
                la_rep = lm4[:, 0].rearrange("p c l -> p c l ()").broadcast_to(
                    (P, cc, 2, 2))                  # (a0,a0,a1,a1)
                lb_rep = lm4[:, 1].rearrange("p c l -> p c () l").broadcast_to(
                    (P, cc, 2, 2))                  # (b0,b1,b0,b1)
                pt = pool.tile([P, cc * 4], f32, tag="pt")
                pt4 = pt[:].rearrange("p (c i l) -> p c i l", i=2, l=2)
                v_.tensor_tensor(pt4, la_rep, lb_rep, MUL)
                S["pt"] = pt

            def s_cast():
                it = pool.tile([P, cc * 4], i32, tag="it")
                if c_cast == "scalar":
                    nc.scalar.copy(it[:], S["pt"][:])  # fp32 -> int32 exact
                else:
                    getattr(nc, c_cast).tensor_copy(it[:], S["pt"][:])
                S["it3"] = it[:].rearrange("p (c k) -> p c k", k=4)

            def s_shq():
                sh = scratch("sh", 4)     # pXY >> 12
                v_.tensor_scalar(sh, S["it3"], 12, None, SHR)
                q = scratch("q", 4)       # pXY mod 4096
                v_.scalar_tensor_tensor(q, sh, -4096.0, S["it3"], MUL, ADD)
                S["sh"], S["q"] = sh, q
                dig = scratch("dig", 4)   # the four 12-bit output digits
                S["dig"] = dig
                if c_d0 == "scalar":
                    nc.scalar.copy(dig[:, :, 0:1], q[:, :, 0:1])
                else:
                    getattr(nc, c_d0).tensor_copy(dig[:, :, 0:1], q[:, :, 0:1])

            def s_add1():
                sh, q = S["sh"], S["q"]
                t1 = scratch("t1")
                g.tensor_tensor(t1, q[:, :, 1:2], q[:, :, 2:3], ADD)
                g.tensor_tensor(t1, t1, sh[:, :, 0:1], ADD)   # digit1 raw <= 12283
                u2 = scratch("u2")
                g.tensor_tensor(u2, sh[:, :, 1:2], sh[:, :, 2:3], ADD)
                g.tensor_tensor(u2, u2, q[:, :, 3:4], ADD)
                S["t1"], S["u2"] = t1, u2

            def s_c1():
                c1 = scratch("c1")
                v_.tensor_scalar(c1, S["t1"], 12, None, SHR)
                v_.scalar_tensor_tensor(S["dig"][:, :, 1:2], c1, -4096.0,
                                        S["t1"], MUL, ADD)
                S["c1"] = c1

            def s_add2():
                t2 = scratch("t2")
                g.tensor_tensor(t2, S["u2"], S["c1"], ADD)    # digit2 raw <= 12285
                S["t2"] = t2

            def s_c2():
                c2 = scratch("c2")
                v_.tensor_scalar(c2, S["t2"], 12, None, SHR)
                v_.scalar_tensor_tensor(S["dig"][:, :, 2:3], c2, -4096.0,
                                        S["t2"], MUL, ADD)
                S["c2"] = c2

            def s_d3():
                g.tensor_tensor(S["dig"][:, :, 3:4], S["sh"][:, :, 3:4],
                                S["c2"], ADD)                 # digit3 <= 4095

            def s_band():
                # bits: (digit & (1<<k)); out position = 12*digit + k,
                # contiguous in (c, digit, bit).
                bt = pool.tile([P, cc * 48], i32, tag="bt")
                bt4 = bt[:].rearrange("p (c l b) -> p c l b", l=4, b=12)
                v_.tensor_tensor(bt4, S["dig"].broadcast_to((P, cc, 4, 12)),
                                 mask_bc, AND)
                S["bt"] = bt

            def s_sign():
                ob = pool.tile([P, cc * 48], f32, tag="ob")
                if c_sign == "scalar":
                    nc.scalar.activation(ob[:], S["bt"][:], SIGN)
                else:
                    nc.vector.tensor_scalar(ob[:], S["bt"][:], 0, None,
                                            AluOpType.not_equal)
                S["ob"] = ob

            def s_out():
                # out-DMA off SP: a single engine's DMAs issue in order, so
                # an output blocked on compute would head-of-line-block
                # later loads.
                getattr(nc, c_out).dma_start(
                    Ov[:, rows], S["ob"][:].rearrange("p (c b) -> p c b", b=48))

            return [s_load, s_limbs, s_prod, s_cast, s_shq, s_add1, s_c1,
                    s_add2, s_c2, s_d3, s_band, s_sign, s_out]

        # software-pipelined emission: engine streams are executed in
        # priority (= emission) order, so interleave chunks with a skew to
        # give every engine independent work during cross-engine stalls.
        chunk_stages = []
        row0 = 0
        for ch, cc in enumerate(chunk_sizes):
            chunk_stages.append(make_stages(ch, cc, row0))
            row0 += cc
        n_stages = len(chunk_stages[0])
        n_ch = len(chunk_stages)
        if preload:
            for k in range(n_ch):
                chunk_stages[k][0]()
        for t in range(n_stages + (n_ch - 1) * skew):
            for k in range(n_ch):
                s = t - k * skew
                if (1 if preload else 0) <= s < n_stages:
                    chunk_stages[k][s]()

    nc.compile()
    return nc


_CACHE: dict = {}


def kernel(A: np.ndarray, B: np.ndarray) -> np.ndarray:
    A = np.ascontiguousarray(np.asarray(A, dtype=np.float32))
    B = np.ascontiguousarray(np.asarray(B, dtype=np.float32))
    assert A.shape == (BATCH, 24) and B.shape == (BATCH, 24), (A.shape, B.shape)

    if "nc" not in _CACHE:
        _CACHE["nc"] = build_nc(**DEFAULT_BUILD)
    nc = _CACHE["nc"]

    in_maps = []
    for i in range(N_CORES):
        sl = slice(i * NB, (i + 1) * NB)
        in_maps.append({"A": A[sl], "B": B[sl]})

    res = run_bass_kernel_spmd(nc, in_maps, core_ids=list(range(N_CORES)))
    outs = [np.asarray(res.results[i]["out"]) for i in range(N_CORES)]
    return np.concatenate(outs, axis=0).astype(np.float32)


if __name__ == "__main__":
    rng = np.random.default_rng(0)
    A = rng.integers(0, 2, (BATCH, 24)).astype(np.float32)
    B = rng.integers(0, 2, (BATCH, 24)).astype(np.float32)
    out = kernel(A, B)
    pw = (1 << np.arange(24)).astype(np.int64)
    a = (A.astype(np.int64) * pw).sum(-1)
    b = (B.astype(np.int64) * pw).sum(-1)
    p = a * b
    exp = ((p[:, None] >> np.arange(48)[None, :]) & 1).astype(np.float32)
    print("max abs diff:", np.abs(out - exp).max())
    assert np.array_equal(out, exp), "MISMATCH"
    print("EXACT MATCH")


# revision 55
# speedup vs baseline: 1.0895x; 1.0468x over previous
"""24x24-bit array multiplier on 8 TRN2 NeuronCores (Bass).

A, B: [65536, 24] float32 0/1 bit-vectors (LSB first) -> P: [65536, 48] bits.

Strategy (pure data parallel, batch sharded 8 ways):
  Per core shard = 8192 rows laid out as [128 partitions x 64 rows/partition],
  processed in chunks pipelined across engines:
  1. bits -> 12-bit limb values via a fused shift-add tree
     (scalar_tensor_tensor), A and B side by side in one tile so each
     tree level is a single op; all fp32 exact (values < 4096). [VectorE]
  2. All 4 limb cross-products in one op via repeat-APs, fp32
     exact (< 2^24); cast to int32.                      [VectorE, ScalarE]
  3. Radix-4096 digit-serial combine: shifts/masks on VectorE (the
     only engine walrus allows TensorScalarPtr on), carry adds on
     GpSimd (plain TT adds are Pool-legal). Every add < 2^14 because
     the vector engines run int adds through the fp32 datapath
     (exact only below 2^24); "mod 4096" is the fused arithmetic
     (x>>12)*-4096 + x, exact in fp32.
  4. All 48 bits at once: digits broadcast-ANDed against an
     iota-built 12-mask vector [VectorE], then Sign() -> fp32 0/1
     [ScalarE], DMA out.
"""

import numpy as np

import concourse.bacc as bacc
import concourse.bass as bass
import concourse.mybir as mybir
import concourse.tile as tile
from concourse.alu_op_type import AluOpType
from concourse.bass_utils import run_bass_kernel_spmd

P = 128           # SBUF partitions
C = 64            # batch rows per partition
NB = P * C        # rows per core = 8192
N_CORES = 8
BATCH = NB * N_CORES

f32 = mybir.dt.float32
i32 = mybir.dt.int32

AND, SHR, SHL = (AluOpType.bitwise_and, AluOpType.logical_shift_right,
                 AluOpType.logical_shift_left)
ADD, OR, MUL = AluOpType.add, AluOpType.bitwise_or, AluOpType.mult
SIGN = mybir.ActivationFunctionType.Sign

DEFAULT_BUILD = dict(chunk_sizes=[20, 20, 16, 8], bufs=3, skew=3,
                     out_eng="sync", preload=True, cast_eng="gpsimd",
                     d0_eng="none",
                     chunk_overrides={0: dict(split_tree=True),
                                      2: dict(split_out=True),
                                      3: dict(sign_eng="vector",
                                              split_out=True)})


def build_nc(chunk_sizes: list | None = None, bufs: int = 3,
             cast_eng: str = "scalar", adds_eng: str = "gpsimd",
             sign_eng: str = "scalar", out_eng: str = "scalar",
             d0_eng: str = "scalar", skew: int = 3, preload: bool = False,
             limb_mode: str = "tree", prod_eng: str = "vector",
             q_eng: str = "vector",
             chunk_overrides: dict | None = None) -> bass.Bass:
    """One SPMD program; every core runs it on its own shard."""
    nc = bacc.Bacc(
        "TRN2",
        target_bir_lowering=False,
        debug=False,
        num_devices=N_CORES,
    )
    A = nc.declare_dram_parameter("A", [NB, 24], f32, isOutput=False)
    B = nc.declare_dram_parameter("B", [NB, 24], f32, isOutput=False)
    OUT = nc.declare_dram_parameter("out", [NB, 48], f32, isOutput=True)

    if chunk_sizes is None:
        chunk_sizes = DEFAULT_BUILD["chunk_sizes"]
    assert sum(chunk_sizes) == C

    # DRAM views: row (p*C + c) lives on partition p, slot c.
    Av = A[:].rearrange("(p c) b -> p c b", p=P)
    Bv = B[:].rearrange("(p c) b -> p c b", p=P)
    Ov = OUT[:].rearrange("(p c) b -> p c b", p=P)

    with tile.TileContext(nc) as tc, \
            tc.tile_pool(name="const", bufs=1) as cpool, \
            tc.tile_pool(name="work", bufs=bufs) as pool:
        # mask[p, k] = 1 << k for k in 0..11, built on-chip (no DMA)
        iot = cpool.tile([P, 12], i32)
        nc.gpsimd.iota(iot[:], [[1, 12]], channel_multiplier=0)
        ones = cpool.tile([P, 12], i32)
        nc.vector.memset(ones[:], 1)
        mask_t = cpool.tile([P, 12], i32)
        nc.vector.tensor_tensor(mask_t[:], ones[:], iot[:], SHL)
        mask_r = mask_t[:].rearrange("p b -> p () () b")

        cm4096 = None
        if q_eng == "gpsimd":
            cm4096 = cpool.tile([P, 1], i32)
            nc.vector.memset(cm4096[:], 4096)

        sc0 = None
        if limb_mode == "scan":
            # Horner-scan multiplier stream: 0.5 everywhere, 0 at each
            # 12-bit segment start (resets the recurrence per limb).
            ccm = max(chunk_sizes)
            sc0 = cpool.tile([P, 2 * ccm * 24], f32)
            nc.vector.memset(sc0[:], 0.5)
            sc0v = sc0[:].rearrange("p (c l b) -> p c l b", l=2, b=12)
            nc.vector.memset(sc0v[:, :, :, 0], 0.0)

        def make_stages(ch, cc, row0):
            """Return the list of stage-emitter closures for one chunk."""
            ov = (chunk_overrides or {}).get(ch, {})
            c_cast = ov.get("cast_eng", cast_eng)
            c_adds = ov.get("adds_eng", adds_eng)
            c_sign = ov.get("sign_eng", sign_eng)
            c_out = ov.get("out_eng", out_eng)
            c_d0 = ov.get("d0_eng", d0_eng)
            c_split = ov.get("split_tree", False)
            c_splitout = ov.get("split_out", False)
            rows = slice(row0, row0 + cc)
            mask_bc = mask_r.broadcast_to((P, cc, 4, 12))
            v_ = nc.vector
            g = getattr(nc, c_adds)   # plain adds (Pool-legal)
            S: dict = {}

            def scratch(tg, k=1):
                s = pool.tile([P, cc * k], i32, tag=tg)
                return s[:].rearrange("p (c k) -> p c k", k=k)

            def s_load():
                # A and B side by side: [P, (2, cc, 24)]
                ab = pool.tile([P, 2 * cc * 24], f32, tag="ab")
                S["ab3"] = ab[:].rearrange("p (i c b) -> p i c b", i=2, b=24)
                nc.sync.dma_start(S["ab3"][:, 0], Av[:, rows])
                nc.sync.dma_start(S["ab3"][:, 1], Bv[:, rows])

            def _tree(src3, half):
                """One input's tree: src3 [P, cc, 24] -> lm4[:, half]."""
                u = pool.tile([P, cc * 12], f32, tag=f"u{half}")
                u3 = u[:].rearrange("p (c b) -> p c b", b=12)
                v_.scalar_tensor_tensor(u3, src3[:, :, 1::2], 2.0,
                                        src3[:, :, 0::2], MUL, ADD)
                v = pool.tile([P, cc * 6], f32, tag=f"v{half}")
                v3 = v[:].rearrange("p (c b) -> p c b", b=6)
                v_.scalar_tensor_tensor(v3, u3[:, :, 1::2], 4.0,
                                        u3[:, :, 0::2], MUL, ADD)
                v4 = v[:].rearrange("p (c l b) -> p c l b", l=2, b=3)
                t = pool.tile([P, cc * 2], f32, tag=f"t{half}")
                t3 = t[:].rearrange("p (c l) -> p c l", l=2)
                v_.scalar_tensor_tensor(t3, v4[:, :, :, 1], 16.0,
                                        v4[:, :, :, 0], MUL, ADD)
                v_.scalar_tensor_tensor(S["lm4"][:, half], v4[:, :, :, 2],
                                        256.0, t3, MUL, ADD)

            def s_limbs():
                if limb_mode == "scan":
                    # Horner scan s' = 0.5*s + bit (exact dyadics); cols 11
                    # and 23 of each 24-group hold the limbs scaled 2^-11.
                    so = pool.tile([P, 2 * cc * 24], f32, tag="so")
                    sof = so[:]
                    if c_split:
                        for h in range(2):
                            v_.tensor_tensor_scan(
                                sof[:, h * cc * 24:(h + 1) * cc * 24],
                                sc0[:][:, :cc * 24],
                                S["ab3"][:, h].rearrange("p c b -> p (c b)"),
                                0.0, MUL, ADD)
                    else:
                        v_.tensor_tensor_scan(
                            sof, sc0[:][:, :2 * cc * 24],
                            S["ab3"].rearrange("p i c b -> p (i c b)"),
                            0.0, MUL, ADD)
                    S["lm4"] = so[:].rearrange(
                        "p (i c l b) -> p i c l b", i=2, l=2, b=12)[:, :, :, :, 11]
                    return
                # shift-add tree -> limbs [P, (2, cc, 2)] fp32 (lo12, hi12)
                ab3 = S["ab3"]
                lm = pool.tile([P, 2 * cc * 2], f32, tag="lm")
                S["lm4"] = lm[:].rearrange("p (i c l) -> p i c l", i=2, l=2)
                if c_split:
                    # separate trees: A's can start before B's DMA lands
                    _tree(ab3[:, 0], 0)
                    _tree(ab3[:, 1], 1)
                    return
                u = pool.tile([P, 2 * cc * 12], f32, tag="u")
                u3 = u[:].rearrange("p (i c b) -> p i c b", i=2, b=12)
                v_.scalar_tensor_tensor(u3, ab3[:, :, :, 1::2], 2.0,
                                        ab3[:, :, :, 0::2], MUL, ADD)
                v = pool.tile([P, 2 * cc * 6], f32, tag="v")
                v3 = v[:].rearrange("p (i c b) -> p i c b", i=2, b=6)
                v_.scalar_tensor_tensor(v3, u3[:, :, :, 1::2], 4.0,
                                        u3[:, :, :, 0::2], MUL, ADD)
                v4 = v[:].rearrange("p (i c l b) -> p i c l b", i=2, l=2, b=3)
                t = pool.tile([P, 2 * cc * 2], f32, tag="t")
                t4 = t[:].rearrange("p (i c l) -> p i c l", i=2, l=2)
                v_.scalar_tensor_tensor(t4, v4[:, :, :, :, 1], 16.0,
                                        v4[:, :, :, :, 0], MUL, ADD)
                v_.scalar_tensor_tensor(S["lm4"], v4[:, :, :, :, 2], 256.0,
                                        t4, MUL, ADD)

            def s_prod():
                # all 4 cross products in one op:
                # pt[c, :] = (a0b0, a0b1, a1b0, a1b1)
                lm4 = S["lm4"]
                la_rep = lm4[:, 0].rearrange("p c l -> p c l ()").broadcast_to(
                    (P, cc, 2, 2))                  # (a0,a0,a1,a1)
                lb_rep = lm4[:, 1].rearrange("p c l -> p c () l").broadcast_to(
                    (P, cc, 2, 2))                  # (b0,b1,b0,b1)
                pt = pool.tile([P, cc * 4], f32, tag="pt")
                pt4 = pt[:].rearrange("p (c i l) -> p c i l", i=2, l=2)
                if limb_mode == "scan":
                    # limbs carry 2^-11 each; (la*2^22)*lb restores pXY,
                    # exact: la*2^22 = a_limb*2^11 < 2^23.
                    v_.scalar_tensor_tensor(pt4, la_rep, float(2 ** 22),
                                            lb_rep, MUL, MUL)
                else:
                    getattr(nc, ov.get("prod_eng", prod_eng)).tensor_tensor(
                        pt4, la_rep, lb_rep, MUL)
                S["pt"] = pt

            def s_cast():
                it = pool.tile([P, cc * 4], i32, tag="it")
                if c_cast == "scalar":
                    nc.scalar.copy(it[:], S["pt"][:])  # fp32 -> int32 exact
                else:
                    getattr(nc, c_cast).tensor_copy(it[:], S["pt"][:])
                S["it3"] = it[:].rearrange("p (c k) -> p c k", k=4)

            def s_shq():
                sh = scratch("sh", 4)     # pXY >> 12
                v_.tensor_scalar(sh, S["it3"], 12, None, SHR)
                q = scratch("q", 4)       # pXY mod 4096
                if ov.get("q_eng", q_eng) == "gpsimd":
                    m = scratch("m", 4)
                    cm_bc = cm4096[:].rearrange("p o -> p () o").broadcast_to(
                        (P, cc, 4))
                    g.tensor_tensor(m, sh, cm_bc, MUL)
                    g.tensor_tensor(q, S["it3"], m, AluOpType.subtract)
                else:
                    v_.scalar_tensor_tensor(q, sh, -4096.0, S["it3"], MUL, ADD)
                S["sh"], S["q"] = sh, q
                dig = scratch("dig", 4)   # the four 12-bit output digits
                S["dig"] = dig
                if c_d0 == "none":
                    pass  # band reads digit0 straight from the q tile
                elif c_d0 == "scalar":
                    nc.scalar.copy(dig[:, :, 0:1], q[:, :, 0:1])
                else:
                    getattr(nc, c_d0).tensor_copy(dig[:, :, 0:1], q[:, :, 0:1])

            # The band only examines bits 0..11 of each digit (masks are
            # 1<<0 .. 1<<11), so the raw carry sums t1 <= 12283 and
            # t2 <= 12285 serve as digits directly — no "mod 4096" ops.
            # Pool's final adds write straight into the dig tile.
            def s_add1():
                sh, q = S["sh"], S["q"]
                t1a = scratch("t1a")
                g.tensor_tensor(t1a, q[:, :, 1:2], q[:, :, 2:3], ADD)
                g.tensor_tensor(S["dig"][:, :, 1:2], t1a, sh[:, :, 0:1],
                                ADD)                          # digit1 raw <= 12283
                u2 = scratch("u2")
                g.tensor_tensor(u2, sh[:, :, 1:2], sh[:, :, 2:3], ADD)
                g.tensor_tensor(u2, u2, q[:, :, 3:4], ADD)
                S["u2"] = u2

            def s_c1():
                c1 = scratch("c1")
                v_.tensor_scalar(c1, S["dig"][:, :, 1:2], 12, None, SHR)
                S["c1"] = c1

            def s_add2():
                g.tensor_tensor(S["dig"][:, :, 2:3], S["u2"], S["c1"],
                                ADD)                          # digit2 raw <= 12285

            def s_c2():
                c2 = scratch("c2")
                v_.tensor_scalar(c2, S["dig"][:, :, 2:3], 12, None, SHR)
                S["c2"] = c2

            def s_d3():
                g.tensor_tensor(S["dig"][:, :, 3:4], S["sh"][:, :, 3:4],
                                S["c2"], ADD)                 # digit3 <= 4095

            def s_band():
                # bits: (digit & (1<<k)); out position = 12*digit + k,
                # contiguous in (c, digit, bit).
                bt = pool.tile([P, cc * 48], i32, tag="bt")
                bt4 = bt[:].rearrange("p (c l b) -> p c l b", l=4, b=12)
                if c_d0 == "none":
                    v_.tensor_tensor(
                        bt4[:, :, 0:1],
                        S["q"][:, :, 0:1].broadcast_to((P, cc, 1, 12)),
                        mask_r.broadcast_to((P, cc, 1, 12)), AND)
                    v_.tensor_tensor(
                        bt4[:, :, 1:4],
                        S["dig"][:, :, 1:4].broadcast_to((P, cc, 3, 12)),
                        mask_r.broadcast_to((P, cc, 3, 12)), AND)
                else:
                    v_.tensor_tensor(bt4,
                                     S["dig"].broadcast_to((P, cc, 4, 12)),
                                     mask_bc, AND)
                S["bt"] = bt

            def _sign_piece(dst, src):
                if c_sign == "scalar":
                    nc.scalar.activation(dst, src, SIGN)
                else:
                    nc.vector.tensor_scalar(dst, src, 0, None,
                                            AluOpType.not_equal)

            def s_sign():
                ob = pool.tile([P, cc * 48], f32, tag="ob")
                S["ob"] = ob
                if not c_splitout:
                    _sign_piece(ob[:], S["bt"][:])
                    return
                h = cc // 2
                _sign_piece(ob[:][:, :h * 48], S["bt"][:][:, :h * 48])
                getattr(nc, c_out).dma_start(
                    Ov[:, rows.start:rows.start + h],
                    ob[:][:, :h * 48].rearrange("p (c b) -> p c b", b=48))
                _sign_piece(ob[:][:, h * 48:], S["bt"][:][:, h * 48:])

            def s_out():
                # out-DMA off SP: a single engine's DMAs issue in order, so
                # an output blocked on compute would head-of-line-block
                # later loads.
                if c_splitout:
                    h = cc // 2
                    getattr(nc, c_out).dma_start(
                        Ov[:, rows.start + h:rows.stop],
                        S["ob"][:][:, h * 48:].rearrange("p (c b) -> p c b", b=48))
                    return
                getattr(nc, c_out).dma_start(
                    Ov[:, rows], S["ob"][:].rearrange("p (c b) -> p c b", b=48))

            return [s_load, s_limbs, s_prod, s_cast, s_shq, s_add1, s_c1,
                    s_add2, s_c2, s_d3, s_band, s_sign, s_out]

        # software-pipelined emission: engine streams are executed in
        # priority (= emission) order, so interleave chunks with a skew to
        # give every engine independent work during cross-engine stalls.
        chunk_stages = []
        row0 = 0
        for ch, cc in enumerate(chunk_sizes):
            chunk_stages.append(make_stages(ch, cc, row0))
            row0 += cc
        n_stages = len(chunk_stages[0])
        n_ch = len(chunk_stages)
        if preload:
            for k in range(n_ch):
                chunk_stages[k][0]()
        for t in range(n_stages + (n_ch - 1) * skew):
            for k in range(n_ch):
                s = t - k * skew
                if (1 if preload else 0) <= s < n_stages:
                    chunk_stages[k][s]()

    nc.compile()
    return nc


_CACHE: dict = {}


def kernel(A: np.ndarray, B: np.ndarray) -> np.ndarray:
    A = np.ascontiguousarray(np.asarray(A, dtype=np.float32))
    B = np.ascontiguousarray(np.asarray(B, dtype=np.float32))
    assert A.shape == (BATCH, 24) and B.shape == (BATCH, 24), (A.shape, B.shape)

    if "nc" not in _CACHE:
        _CACHE["nc"] = build_nc(**DEFAULT_BUILD)
    nc = _CACHE["nc"]

    in_maps = []
    for i in range(N_CORES):
        sl = slice(i * NB, (i + 1) * NB)
        in_maps.append({"A": A[sl], "B": B[sl]})

    res = run_bass_kernel_spmd(nc, in_maps, core_ids=list(range(N_CORES)))
    outs = [np.asarray(res.results[i]["out"]) for i in range(N_CORES)]
    return np.concatenate(outs, axis=0).astype(np.float32)


if __name__ == "__main__":
    rng = np.random.default_rng(0)
    A = rng.integers(0, 2, (BATCH, 24)).astype(np.float32)
    B = rng.integers(0, 2, (BATCH, 24)).astype(np.float32)
    out = kernel(A, B)
    pw = (1 << np.arange(24)).astype(np.int64)
    a = (A.astype(np.int64) * pw).sum(-1)
    b = (B.astype(np.int64) * pw).sum(-1)
    p = a * b
    exp = ((p[:, None] >> np.arange(48)[None, :]) & 1).astype(np.float32)
    print("max abs diff:", np.abs(out - exp).max())
    assert np.array_equal(out, exp), "MISMATCH"
    print("EXACT MATCH")
